# revision 30
# baseline (speedup 1.0000x reference)
"""ClusterAttention TRN2 kernel.

Computation (per batch item):
  a  = conv1d_same(x, w1, b1)                      # (k, p) conv over p
  bm = conv1d_same(x^T, w2, b2)                    # (k, c) conv over c
  bm = bm @ x                                      # (k, p)
  probs = softmax(a + bm, axis=k)                  # (k, p)
  out = w_post @ (probs[:, None, :] * x[None]).reshape(k*c, p) + b_post

Sharding: data-parallel over batch n (32) across 8 cores, 4 per core.
All params replicated.

Schedule (per core): software-pipelined across the 4 batch items.  All
engines are in-order, so emission order is arranged so nothing the PE
needs ever sits behind later work in another engine's queue:

  front(b):  x load, bf16 copy, x transposes, A/BM conv matmuls (PE)
  tree(b):   tap-merge shift-add tree on Pool (gpsimd), runs entirely
             under main(b-1); DVE for b==0 (startup)
  fillers inside main(b-1)'s kk loop: bm-merge + logit-assembly PE
             bits (bias adds on Pool, which idle-waits)
  back_sm(b): softmax: 16 PE transposes into one wide psum tile, wide
             DVE middle (max/sub/exp/sum/recip/mul), h0 transpose-back
  front(b+1), back_sm2(b) (h1 transpose-back), then main(b)

Main matmul y = probs (x) x is built via DMA partition-broadcast of
probs rows bounced through DRAM plus DVE bf16 multiplies.  DMA queues:
ACT carries weights/x loads/out stores, SP carries probs stores and
broadcasts, so neither blocks the other's critical path.
"""
import sys

sys.path.insert(0, "/opt/trn_rl_repo")

import numpy as np

import concourse.bass as bass
import concourse.mybir as mybir
import concourse.tile as tile
from concourse import bacc
from concourse.bass_utils import run_bass_kernel_spmd

dt = mybir.dt

N_CORES = 8
B = 4            # batch items per core
C = 256          # channels
P = 2048         # sequence length
K = 16           # clusters
TAPS = 15
PAD = 7
CT = C // 128    # c-tiles
NCH = P // 512   # 512-wide p-chunks
PW = P // 128    # 128-wide p-windows
HALF = P // 2

AW = P + 16      # A_buf width: col j holds A[p = j - PAD], zeros outside
BMW = C + 16     # BM_buf width

F32R = dt.float32r


def build_nc():
    nc = bacc.Bacc(None)

    xs_d = nc.dram_tensor("xs", [B, C, P], dt.float32, kind="ExternalInput")
    w1r_d = nc.dram_tensor("w1r", [C, 256], dt.float32, kind="ExternalInput")
    w2r_d = nc.dram_tensor("w2r", [P, 256], dt.float32, kind="ExternalInput")
    wpt_d = nc.dram_tensor("wpt", [K * C, C], dt.bfloat16, kind="ExternalInput")
    b1_d = nc.dram_tensor("b1c", [K, 1], dt.float32, kind="ExternalInput")
    b2_d = nc.dram_tensor("b2c", [K, 1], dt.float32, kind="ExternalInput")
    bp_d = nc.dram_tensor("bpc", [C, 1], dt.float32, kind="ExternalInput")
    id_d = nc.dram_tensor("ident", [128, 128], dt.float32, kind="ExternalInput")
    sel_d = nc.dram_tensor("sel", [32, 32], dt.float32, kind="ExternalInput")
    probs_d = nc.dram_tensor("probsd", [2, K, P], dt.bfloat16, kind="Internal")
    out_d = nc.dram_tensor("out", [B, C, P], dt.float32, kind="ExternalOutput")

    with tile.TileContext(nc) as tc:
        with (
            tc.tile_pool(name="const", bufs=1) as cpool,
            tc.tile_pool(name="xs", bufs=2) as xpool,
            tc.tile_pool(name="xt", bufs=1) as xtpool,
            tc.tile_pool(name="abuf", bufs=1) as apool,
            tc.tile_pool(name="tree", bufs=1) as tpool,
            tc.tile_pool(name="sm", bufs=2) as smpool,
            tc.tile_pool(name="probs", bufs=1) as ppool,
            tc.tile_pool(name="bc", bufs=2) as bcpool,
            tc.tile_pool(name="y", bufs=6) as ypool,
            tc.tile_pool(name="oc", bufs=2) as ocpool,
            tc.tile_pool(name="pconv", bufs=2, space="PSUM") as pconv,
            tc.tile_pool(name="ptr", bufs=2, space="PSUM") as ptr,
            tc.tile_pool(name="pout", bufs=1, space="PSUM") as pout,
        ):
            # ---- conv-critical consts, then x0, rest of weights ----
            ident = cpool.tile([128, 128], F32R, tag="ident")
            nc.scalar.dma_start(out=ident[:], in_=id_d[:].bitcast(F32R))
            w1r = [cpool.tile([128, 256], F32R, tag=f"w1r{i}", name=f"w1r{i}")
                   for i in range(CT)]
            for i in range(CT):
                nc.scalar.dma_start(out=w1r[i][:],
                                    in_=w1r_d[i * 128:(i + 1) * 128, :].bitcast(F32R))

            def load_x(b):
                # 4 chunked loads on the software DGE (Pool) so no single
                # transfer hogs the DMA engines during a main phase
                xt = xpool.tile([128, CT * P], F32R, tag="xst", name=f"xst{b}")
                for q4 in range(4):
                    nc.gpsimd.dma_start(
                        out=xt[:, q4 * 512:].rearrange(
                            "p (i q) -> p i q", i=CT, q=P)[:, :, 0:512]
                        if False else
                        xt[:].rearrange("p (i q) -> p i q", i=CT, q=P)
                        [:, :, q4 * 512:(q4 + 1) * 512],
                        in_=xs_d[b].bitcast(F32R)
                        .rearrange("(i p) q -> p i q", i=CT, p=128)
                        [:, :, q4 * 512:(q4 + 1) * 512])
                return [xt[:, ct * P:(ct + 1) * P] for ct in range(CT)]

            xs_tiles = {}
            xs_tiles[0] = load_x(0)

            sel = cpool.tile([32, 32], F32R, tag="sel")
            nc.scalar.dma_start(out=sel[:], in_=sel_d[:].bitcast(F32R))
            b1c = cpool.tile([K, 1], dt.float32, tag="b1c")
            b2c = cpool.tile([K, 1], dt.float32, tag="b2c")
            nc.scalar.dma_start(out=b1c[:], in_=b1_d[:])
            nc.scalar.dma_start(out=b2c[:], in_=b2_d[:])
            bpc = [cpool.tile([128, 1], dt.float32, tag=f"bpc{i}", name=f"bpc{i}")
                   for i in range(CT)]
            for i in range(CT):
                nc.scalar.dma_start(out=bpc[i][:], in_=bp_d[i * 128:(i + 1) * 128, :])
            w2rt = cpool.tile([128, PW * 256], F32R, tag="w2rt")
            nc.scalar.dma_start(
                out=w2rt[:].rearrange("p (i c) -> p i c", i=PW, c=256),
                in_=w2r_d[:].bitcast(F32R)
                .rearrange("(i p) c -> p i c", i=PW, p=128))
            w2r = [w2rt[:, i * 256:(i + 1) * 256] for i in range(PW)]
            xs_tiles[1] = load_x(1)
            wptt = cpool.tile([128, 2 * K * 256], dt.bfloat16, tag="wptt")
            nc.scalar.dma_start(
                out=wptt[:].rearrange("p (i c) -> p i c", i=2 * K, c=256),
                in_=wpt_d[:].rearrange("(i p) c -> p i c", i=2 * K, p=128))
            wpt = [wptt[:, i * 256:(i + 1) * 256] for i in range(2 * K)]

            st = {}   # per-batch state handles

            def conv_front(b):
                if b + 1 >= 2 and b + 1 < B:
                    xs_tiles[b + 1] = load_x(b + 1)   # prefetch next batch
                xs = xs_tiles[b]

                xT = [xtpool.tile([128, C], F32R, tag=f"xT{pw}",
                                  name=f"xT{b}_{pw}") for pw in range(PW)]
                for pw in range(PW):
                    for ct in range(CT):
                        pt = ptr.tile([128, 128], dt.float32, tag="ptr")
                        nc.tensor.transpose(
                            pt[:].bitcast(F32R),
                            xs[ct][:, pw * 128:(pw + 1) * 128],
                            ident[:])
                        nc.scalar.copy(xT[pw][:, ct * 128:(ct + 1) * 128], pt[:])

                A = [apool.tile([128, AW], dt.float32, tag=f"A{mc}",
                                name=f"A{b}_{mc}") for mc in range(2)]
                if b == 0:
                    for mc in range(2):
                        nc.vector.memset(A[mc][:, 0:PAD], 0.0)
                        nc.vector.memset(A[mc][:, PAD + P:AW], 0.0)
                for mc in range(2):
                    for nch in range(NCH):
                        pa = pconv.tile([128, 512], dt.float32, tag="pconv")
                        for ct in range(CT):
                            nc.tensor.matmul(
                                pa[:],
                                w1r[ct][:, mc * 128:(mc + 1) * 128],
                                xs[ct][:, nch * 512:(nch + 1) * 512],
                                start=(ct == 0), stop=(ct == CT - 1))
                        nc.scalar.copy(
                            A[mc][:, PAD + nch * 512:PAD + (nch + 1) * 512], pa[:])

                BM = [apool.tile([128, BMW], dt.float32, tag=f"BM{mc}",
                                 name=f"BM{b}_{mc}") for mc in range(2)]
                if b == 0:
                    for mc in range(2):
                        nc.vector.memset(BM[mc][:, 0:PAD], 0.0)
                        nc.vector.memset(BM[mc][:, PAD + C:BMW], 0.0)
                for mc in range(2):
                    pb = pconv.tile([128, C], dt.float32, tag="pconv")
                    for pw in range(PW):
                        nc.tensor.matmul(
                            pb[:],
                            w2r[pw][:, mc * 128:(mc + 1) * 128],
                            xT[pw][:, 0:C],
                            start=(pw == 0), stop=(pw == PW - 1))
                    nc.scalar.copy(BM[mc][:, PAD:PAD + C], pb[:])

                # bf16 x copy last: keeps the in-order ACT queue free for the
                # xT/A/BM copies that gate the Pool tree
                xbt = xpool.tile([128, CT * P], dt.bfloat16, tag="xbt",
                                 bufs=1, name=f"xbt{b}")
                for ct in range(CT):
                    nc.scalar.copy(xbt[:, ct * P:(ct + 1) * P],
                                   xs[ct][:].bitcast(dt.float32))

                st[b] = {"xs": xs, "xb": xbt, "A": A, "BM": BM}

            def tree(b):
                """Tap-merge trees. Pool for b>0 (runs under main(b-1));
                batch 0 splits halves across DVE and Pool for startup."""
                A, BM = st[b]["A"], st[b]["BM"]
                Da = []
                for h in range(2):
                    eng = (nc.vector if h == 0 else nc.gpsimd) \
                        if b == 0 else nc.gpsimd
                    o = h * HALF
                    w1_, w2_, w3_ = HALF + 8, HALF + 4, HALF + 2
                    Bl = tpool.tile([64, w1_], dt.float32, tag="Bl",
                                    name=f"Bl{b}_{h}")
                    Bh = tpool.tile([64, w1_], dt.float32, tag="Bh",
                                    name=f"Bh{b}_{h}")
                    eng.tensor_add(Bl[:], A[0][0:64, o:o + w1_],
                                   A[1][0:64, o + 8:o + 8 + w1_])
                    eng.tensor_add(Bh[:], A[0][64:128, o:o + w1_],
                                   A[1][64:128, o + 8:o + 8 + w1_])
                    Cl = tpool.tile([32, w2_], dt.float32, tag="Cl",
                                    name=f"Cl{b}_{h}")
                    Ch = tpool.tile([32, w2_], dt.float32, tag="Ch",
                                    name=f"Ch{b}_{h}")
                    eng.tensor_add(Cl[:], Bl[0:32, 0:w2_], Bh[0:32, 4:4 + w2_])
                    eng.tensor_add(Ch[:], Bl[32:64, 0:w2_], Bh[32:64, 4:4 + w2_])
                    D = tpool.tile([32, w3_], F32R, tag=f"D{h}",
                                   name=f"D{b}_{h}", bufs=1)
                    eng.tensor_add(D[:], Cl[0:32, 0:w3_], Ch[0:32, 2:2 + w3_])
                    Da.append(D)
                eng = nc.vector if b == 0 else nc.gpsimd
                w1_, w2_, w3_ = C + 8, C + 4, C + 2
                Bl = tpool.tile([64, w1_], dt.float32, tag="Blb", name=f"Blb{b}")
                Bh = tpool.tile([64, w1_], dt.float32, tag="Bhb", name=f"Bhb{b}")
                eng.tensor_add(Bl[:], BM[0][0:64, 0:w1_], BM[1][0:64, 8:8 + w1_])
                eng.tensor_add(Bh[:], BM[0][64:128, 0:w1_],
                               BM[1][64:128, 8:8 + w1_])
                Cl = tpool.tile([32, w2_], dt.float32, tag="Clb", name=f"Clb{b}")
                Ch = tpool.tile([32, w2_], dt.float32, tag="Chb", name=f"Chb{b}")
                eng.tensor_add(Cl[:], Bl[0:32, 0:w2_], Bh[0:32, 4:4 + w2_])
                eng.tensor_add(Ch[:], Bl[32:64, 0:w2_], Bh[32:64, 4:4 + w2_])
                Db = tpool.tile([32, w3_], F32R, tag="Db", name=f"Db{b}", bufs=1)
                eng.tensor_add(Db[:], Cl[0:32, 0:w3_], Ch[0:32, 2:2 + w3_])
                st[b]["Da"] = Da
                st[b]["Db"] = Db

            def bm_merge_pe(b):
                """sel-merge of bm tree + bias (bias on ACT)."""
                Db = st[b]["Db"]
                pb4 = pconv.tile([128, 512], dt.float32, tag="pconv",
                                 name=f"pb4_{b}")
                nc.tensor.matmul(pb4[0:K, 0:C], sel[:, 0:16], Db[:, 0:C],
                                 start=True, stop=False)
                nc.tensor.matmul(pb4[0:K, 0:C], sel[:, 16:32], Db[:, 1:1 + C],
                                 start=False, stop=True)
                bmt = smpool.tile([K, C], F32R, tag="bmt", bufs=1, name=f"bmt{b}")
                nc.scalar.add(bmt[:], pb4[0:K, 0:C], b2c[:])
                st[b]["bmt"] = bmt

            def bm_transpose(b):
                bmt = st[b]["bmt"]
                bmT = [smpool.tile([128, K], F32R, tag=f"bmT{ct}",
                                   name=f"bmT{b}_{ct}", bufs=1) for ct in range(CT)]
                for ct in range(CT):
                    pt = ptr.tile([128, 128], dt.float32, tag="ptr")
                    nc.tensor.transpose(
                        pt[0:128, 0:K].bitcast(F32R),
                        bmt[:, ct * 128:(ct + 1) * 128],
                        ident[0:K, 0:K])
                    nc.scalar.copy(bmT[ct][:], pt[0:128, 0:K])
                st[b]["bmT"] = bmT

            def logit_piece(b, nch):
                """One 512-wide logit chunk: sel(a-tree) + bmT@x + b1."""
                if nch == 0:
                    st[b]["logit"] = smpool.tile([K, P], F32R, tag="logit",
                                                 bufs=1, name=f"logit{b}")
                logit = st[b]["logit"]
                D = st[b]["Da"][nch // 2]
                bmT = st[b]["bmT"]
                xs = st[b]["xs"]
                o = (nch % 2) * 512
                pa4 = pconv.tile([128, 512], dt.float32, tag="pconv",
                                 name=f"pa4_{b}_{nch}")
                nc.tensor.matmul(pa4[0:K, :], sel[:, 0:16], D[:, o:o + 512],
                                 start=True, stop=False)
                nc.tensor.matmul(pa4[0:K, :], sel[:, 16:32], D[:, o + 1:o + 513],
                                 start=False, stop=False)
                for ct in range(CT):
                    nc.tensor.matmul(
                        pa4[0:K, :], bmT[ct][:],
                        xs[ct][:, nch * 512:(nch + 1) * 512],
                        start=False, stop=(ct == CT - 1))
                nc.scalar.add(
                    logit[:, nch * 512:(nch + 1) * 512], pa4[0:K, :], b1c[:])

            def back_sm(b):
                """Softmax: wide transposes + wide DVE middle + h0 back."""
                logit = st[b]["logit"]
                W = K * PW  # 256
                plT = pconv.tile([128, 512], dt.float32, tag="pconv",
                                 name=f"plT{b}")
                for pw in range(PW):
                    nc.tensor.transpose(
                        plT[0:128, pw * K:(pw + 1) * K].bitcast(F32R),
                        logit[:, pw * 128:(pw + 1) * 128],
                        ident[0:K, 0:K])
                negmx = smpool.tile([128, PW], dt.float32, tag="negmx", bufs=1)
                nc.vector.tensor_reduce(
                    negmx[:],
                    plT[0:128, 0:W].rearrange("p (g k) -> p g k", g=PW, k=K),
                    axis=mybir.AxisListType.X, op=mybir.AluOpType.max,
                    negate=True)
                sh = smpool.tile([128, W], dt.float32, tag="sh", bufs=1)
                nc.vector.tensor_add(
                    sh[:].rearrange("p (g k) -> p g k", g=PW, k=K),
                    plT[0:128, 0:W].rearrange("p (g k) -> p g k", g=PW, k=K),
                    negmx[:].unsqueeze(2).broadcast_to((128, PW, K)))
                expT = smpool.tile([128, W], dt.float32, tag="expT", bufs=1)
                nc.scalar.activation(
                    expT[:], sh[:], mybir.ActivationFunctionType.Exp)
                esum = smpool.tile([128, PW], dt.float32, tag="esum", bufs=1)
                nc.vector.tensor_reduce(
                    esum[:],
                    expT[:].rearrange("p (g k) -> p g k", g=PW, k=K),
                    axis=mybir.AxisListType.X, op=mybir.AluOpType.add)
                recip = smpool.tile([128, PW], dt.float32, tag="recip", bufs=1)
                nc.vector.reciprocal(recip[:], esum[:])
                probsT = smpool.tile([128, W], F32R, tag="probsT", bufs=1)
                nc.vector.tensor_mul(
                    probsT[:].rearrange("p (g k) -> p g k", g=PW, k=K),
                    expT[:].rearrange("p (g k) -> p g k", g=PW, k=K),
                    recip[:].unsqueeze(2).broadcast_to((128, PW, K)))
                st[b]["probsT"] = probsT
                st[b]["probs"] = ppool.tile([K, P], dt.bfloat16, tag="probs",
                                            bufs=1, name=f"probs{b}")

            def _probs_half(b, h):
                """Transpose probsT half h back to [K, P/2], stream to DRAM."""
                par = b % 2
                probsT = st[b]["probsT"]
                probs = st[b]["probs"]
                for g in range(2 * h, 2 * h + 2):
                    ppg = pconv.tile([128, 512], dt.float32, tag="pconv",
                                     name=f"ppg{b}_{g}")
                    for j in range(4):
                        pw = g * 4 + j
                        nc.tensor.transpose(
                            ppg[0:K, j * 128:(j + 1) * 128].bitcast(F32R),
                            probsT[:, pw * K:(pw + 1) * K],
                            ident[:])
                    nc.vector.tensor_copy(probs[:, g * 512:(g + 1) * 512],
                                          ppg[0:K, 0:512])
                    nc.sync.dma_start(
                        out=probs_d[par, :, g * 512:(g + 1) * 512],
                        in_=probs[:, g * 512:(g + 1) * 512])

            def prefetch_main(b):
                par = b % 2
                xb = st[b]["xb"]
                bct = bcpool.tile([128, 4 * 1024], dt.bfloat16, tag="bct",
                                  name=f"bct{b}_0_0")
                nc.sync.dma_start(
                    out=bct[:].rearrange("p (i q) -> p i q", i=4, q=1024),
                    in_=probs_d[par, 0:4, 0:1024]
                    .unsqueeze(0).broadcast_to((128, 4, 1024)))
                ys = []
                for kk in range(4):
                    y = ypool.tile([128, CT * 1024], dt.bfloat16, tag="y",
                                   name=f"y{b}_0_{kk}")
                    nc.vector.tensor_mul(
                        y[:].rearrange("p (i q) -> p i q", i=CT, q=1024),
                        xb[:].rearrange("p (i q) -> p i q", i=CT, q=P)
                        [:, :, 0:1024],
                        bct[:, kk * 1024:(kk + 1) * 1024]
                        .unsqueeze(1).broadcast_to((128, CT, 1024)))
                    ys.append(y)
                st[b]["pre"] = ys

            def main_phase(b, fillers=None):
                par = b % 2
                xb = st[b]["xb"]
                fillers = fillers or {}
                po = {}
                for half in range(2):
                    for nch in (2 * half, 2 * half + 1):
                        for ot in range(CT):
                            po[(nch, ot)] = pout.tile(
                                [128, 512], dt.float32,
                                tag=f"po{nch % 2}{ot}", name=f"po{b}_{nch}_{ot}")
                    for kk in range(K):
                        pre = half == 0 and kk < 4 and "pre" in st[b]
                        if pre:
                            y = st[b]["pre"][kk]
                        else:
                            if kk % 4 == 0:
                                # one broadcast DMA covers 4 clusters
                                bct = bcpool.tile([128, 4 * 1024], dt.bfloat16,
                                                  tag="bct",
                                                  name=f"bct{b}_{half}_{kk}")
                                nc.sync.dma_start(
                                    out=bct[:].rearrange("p (i q) -> p i q",
                                                         i=4, q=1024),
                                    in_=probs_d[par, kk:kk + 4,
                                                half * 1024:(half + 1) * 1024]
                                    .unsqueeze(0).broadcast_to((128, 4, 1024)))
                            y = ypool.tile([128, CT * 1024], dt.bfloat16, tag="y",
                                           name=f"y{b}_{half}_{kk}")
                            nc.vector.tensor_mul(
                                y[:].rearrange("p (i q) -> p i q", i=CT, q=1024),
                                xb[:].rearrange("p (i q) -> p i q", i=CT, q=P)
                                [:, :, half * 1024:(half + 1) * 1024],
                                bct[:, (kk % 4) * 1024:(kk % 4 + 1) * 1024]
                                .unsqueeze(1).broadcast_to((128, CT, 1024)))
                        for ct in range(CT):
                            for ot in range(CT):
                                for nch in (2 * half, 2 * half + 1):
                                    co = ct * 1024 + (nch % 2) * 512
                                    nc.tensor.matmul(
                                        po[(nch, ot)][:],
                                        wpt[kk * 2 + ct][:, ot * 128:(ot + 1) * 128],
                                        y[:, co:co + 512],
                                        start=(kk == 0 and ct == 0),
                                        stop=(kk == K - 1 and ct == CT - 1))
                        slot = half * K + kk
                        if slot in fillers:
                            for fn in fillers[slot]:
                                fn()
                    # per-half epilogue: bias + store while the other half
                    # (or the next conv front) runs
                    for nch in (2 * half, 2 * half + 1):
                        oc = ocpool.tile([128, 1024], dt.float32, tag="oc",
                                         name=f"oc{b}_{nch}")
                        for ot in range(CT):
                            nc.scalar.add(
                                oc[:, ot * 512:(ot + 1) * 512],
                                po[(nch, ot)][:], bpc[ot][:])
                        steng = nc.sync if nch % 2 == 0 else nc.scalar
                        steng.dma_start(
                            out=out_d[b, :, nch * 512:(nch + 1) * 512]
                            .rearrange("(i p) q -> p i q", i=CT, p=128),
                            in_=oc[:].rearrange("p (i q) -> p i q", i=CT, q=512))

            def make_fillers(b):
                return {
                    K + 0: [lambda: bm_merge_pe(b)],
                    K + 2: [lambda: bm_transpose(b)],
                    K + 4: [lambda: logit_piece(b, 0)],
                    K + 5: [lambda: logit_piece(b, 1)],
                    K + 6: [lambda: logit_piece(b, 2)],
                    K + 7: [lambda: logit_piece(b, 3)],
                    K + 10: [lambda: back_sm(b)],
                    K + 12: [lambda: _probs_half(b, 0)],
                    K + 14: [lambda: _probs_half(b, 1)],
                    K + 15: [lambda: prefetch_main(b)],
                }

            # ---------------- emission schedule ----------------
            conv_front(0)
            tree(0)
            bm_merge_pe(0)
            bm_transpose(0)
            for nch in range(NCH):
                logit_piece(0, nch)
            back_sm(0)
            _probs_half(0, 0)
            _probs_half(0, 1)
            prefetch_main(0)
            conv_front(1)

            tree(1)
            main_phase(0, make_fillers(1))
            conv_front(2)

            tree(2)
            main_phase(1, make_fillers(2))
            conv_front(3)

            tree(3)
            main_phase(2, make_fillers(3))

            main_phase(3)

    nc.compile()
    return nc


_NC_CACHE = None


def _get_nc():
    global _NC_CACHE
    if _NC_CACHE is None:
        _NC_CACHE = build_nc()
    return _NC_CACHE


def prep_inputs(x, w1, b1, w2, b2, w_post, b_post):
    """Host-side rearrangement of weights; returns per-core in_maps."""
    x = np.asarray(x, dtype=np.float32)
    w1r = np.ascontiguousarray(
        np.asarray(w1, np.float32).transpose(1, 2, 0).reshape(C, TAPS * K))
    w1r = np.concatenate([w1r, np.zeros((C, 256 - TAPS * K), np.float32)], axis=1)
    w2r = np.ascontiguousarray(
        np.asarray(w2, np.float32).transpose(1, 2, 0).reshape(P, TAPS * K))
    w2r = np.concatenate([w2r, np.zeros((P, 256 - TAPS * K), np.float32)], axis=1)
    import ml_dtypes
    wpt = np.ascontiguousarray(
        np.asarray(w_post, np.float32).T).astype(ml_dtypes.bfloat16)
    b1c = np.asarray(b1, np.float32).reshape(K, 1)
    b2c = np.asarray(b2, np.float32).reshape(K, 1)
    bpc = np.asarray(b_post, np.float32).reshape(C, 1)
    ident = np.eye(128, dtype=np.float32)
    sel = np.zeros((32, 32), np.float32)
    for i in range(16):
        sel[i, i] = 1.0
        sel[16 + i, 16 + i] = 1.0
    consts = {"w1r": w1r, "w2r": w2r, "wpt": wpt, "b1c": b1c, "b2c": b2c,
              "bpc": bpc, "ident": ident, "sel": sel}
    in_maps = []
    for core in range(N_CORES):
        m = dict(consts)
        m["xs"] = np.ascontiguousarray(x[core * B:(core + 1) * B])
        in_maps.append(m)
    return in_maps


def run(inputs, trace=False):
    import os
    if not trace:
        # the axon NTFF profile hook is unavailable in this container; a
        # stray BASS_TRACE=1 in the environment would crash the run
        os.environ["BASS_NEVER_TRACE"] = "1"
    nc = _get_nc()
    in_maps = prep_inputs(**inputs)
    res = run_bass_kernel_spmd(nc, in_maps, list(range(N_CORES)), trace=trace)
    out = np.concatenate([res.results[i]["out"] for i in range(N_CORES)], axis=0)
    return out.astype(np.float32), res


def kernel(x, w1, b1, w2, b2, w_post, b_post):
    out, _ = run(dict(x=x, w1=w1, b1=b1, w2=w2, b2=b2,
                      w_post=w_post, b_post=b_post))
    return out
